# revision 1
# baseline (speedup 1.0000x reference)
"""Trainium2 Bass kernel for nn_EnhancedFeatureLayer (OHLCV feature extraction
+ per-instance normalization over the sequence axis).

Input : x [131072, 24, 5] fp32 (open, high, low, close, volume)
Output:   [131072, 24, 25] fp32 (25 features, instance-normalized over s)

Strategy (pure data parallel over 8 NeuronCores, 16384 batches each):
  - Load batches b-on-partitions (contiguous DMA), blocks of 2048 batches.
  - Compute log/gain/loss/sign/vmult in b-on-P (free-axis shifts are free),
    pack them with close/vol + a ones column, PE-transpose to S-on-P.
  - One accumulating matmul pair per 128-batch chunk computes every linear
    sequence map (EMAs, moving averages, diffs, cumsum, +eps biases via the
    ones row) with the DATA as the stationary operand, so the output lands
    back in batch-on-partition orientation (the transpose back is free).
  - Nonlinear features (ratios, rsi, abs, log1p) via DVE/GPSIMD/ACT ops.
  - Stats: ACT square + DVE tensor_reduce; rsqrt via exp(-0.5*ln).
  - Affine normalize with stride-0 broadcast APs, split across DVE + GPSIMD.
  - Staging tile [128, (16, 600)] is written in the exact DRAM layout
    (s-major, channel-minor) so the store DMA is 2400B-contiguous per batch.
"""

import math
import sys

import numpy as np

for _p in ("/opt/trn_rl_repo",):
    if _p not in sys.path:
        sys.path.insert(0, _p)

EPS = 1e-8
IN_EPS = 1e-5
S = 24
CIN = 5
COUT = 25
PB = 128          # batches per chunk (partition dim)
G = 8             # chunks per block
BLOCK = PB * G    # 1024 batches
NCORES = 8

NMM = 336         # matmul output columns per chunk
# matmul column layout:
#   0:24    ema3s      24:48   ema6s     48:72   ema12s(+eps)  72:96  ema24s
#   96:120  vol6s      120:144 vol12s
#   144:168 avgG       168:192 avgL(+eps)        192:216 avgS(g+l+eps)
#   216:240 vol(copy)  240:264 returns   264:312 mom3/6 interleaved  312:336 obv
F32R = True       # single-pass fp32r matmuls (producers round via ACT copies)
ACT_SET = "natural_log_exp_and_others"   # one table set covers every func used


# --------------------------------------------------------------------------
# host-side constants
# --------------------------------------------------------------------------

def _ema_mat(span):
    """[24, 24]; column s holds the weights over close[0..s]."""
    a = 2.0 / (span + 1)
    pows = (1.0 - a) ** np.arange(S, dtype=np.float64)
    W = np.zeros((S, S))
    for s in range(S):
        W[: s + 1, s] = pows[: s + 1] / pows[: s + 1].sum()
    return W


def _mavg_pad_mat():
    """[23, 24]; col s = replicate-padded 14-window avg at index max(s-1, 0)."""
    M = np.zeros((23, 23))
    for l in range(23):
        M[0, l] += max(13 - l, 0) / 14.0
        for j in range(max(0, l - 13), l + 1):
            M[j, l] += 1.0 / 14.0
    P = np.zeros((23, 24))
    P[:, 0] = M[:, 0]
    for s in range(1, 24):
        P[:, s] = M[:, s - 1]
    return P


def _build_weights():
    """W1 [49, NMM] applied to [close(24); vol(24); 1];
    W2 [94, NMM] applied to [LC(24); gain(23); loss(23); vmult(23); 1]."""
    W1 = np.zeros((49, NMM))
    W2 = np.zeros((94, NMM))
    E = {k: _ema_mat(k) for k in (3, 6, 12, 24)}
    W1[0:24, 0:24] = E[3]
    W1[0:24, 24:48] = E[6]
    W1[0:24, 48:72] = E[12]
    W1[0:24, 72:96] = E[24]
    W1[24:48, 96:120] = E[6]
    W1[24:48, 120:144] = E[12]
    W1[48, 48:72] = EPS             # ema12s + eps (denominator of ch20)
    P = _mavg_pad_mat()
    W2[24:47, 144:168] = P          # avgG from gain
    W2[47:70, 168:192] = P          # avgL from loss
    W2[93, 168:192] = EPS           # avgL + eps
    W2[24:47, 192:216] = P          # avgS = g-part
    W2[47:70, 192:216] += P         #        + l-part
    W2[93, 192:216] = EPS           #        + eps
    W1[24:48, 216:240] = np.eye(24)  # vol passthrough (for rv = exp(-ln(vol)))
    for s in range(1, 24):          # returns from LC
        W2[s, 240 + s] = 1.0
        W2[s - 1, 240 + s] = -1.0
    for s in range(3, 24):          # mom3
        W2[s, 264 + 2 * s] = 1.0
        W2[s - 3, 264 + 2 * s] = -1.0
    for s in range(6, 24):          # mom6
        W2[s, 265 + 2 * s] = 1.0
        W2[s - 6, 265 + 2 * s] = -1.0
    for s in range(24):             # obv from vmult (rows are s'=1..23)
        W2[70 : 70 + s, 312 + s] = 1.0
    return W1.astype(np.float32), W2.astype(np.float32)


def _hour_consts():
    """Normalized hour_sin / hour_cos, interleaved [s, 2] -> flat [48]."""
    t = np.arange(S, dtype=np.float32)
    ang = (np.float32(2.0 * math.pi) * (t % 24) / np.float32(24.0)).astype(np.float32)
    out = np.empty((S, 2), dtype=np.float32)
    for i, v in enumerate((np.sin(ang).astype(np.float32),
                           np.cos(ang).astype(np.float32))):
        m = v.mean(dtype=np.float32)
        var = v.var(dtype=np.float32)
        out[:, i] = (v - m) / np.sqrt(var + np.float32(IN_EPS))
    return out.reshape(-1)


def _consts():
    W1, W2 = _build_weights()
    idn = np.eye(128, dtype=np.float32)
    hsc = _hour_consts()
    # 24*eps' per channel; channel 21 (rsi via u=1/(1+rs)) has a=-100
    epsAB = np.full(25, 24.0 * IN_EPS, dtype=np.float32)
    epsAB[21] = np.float32(24.0 * IN_EPS / 1e4)
    return {"w1": W1, "w2": W2, "idn": idn, "hsc": hsc, "epsab": epsAB}


# --------------------------------------------------------------------------
# kernel body (Tile)
# --------------------------------------------------------------------------

def kernel_body(tc, outs, ins, repeat=1):
    import concourse.bass as bass
    from concourse import mybir

    nc = tc.nc
    f32 = mybir.dt.float32
    f32r = mybir.dt.float32r
    AF = mybir.ActivationFunctionType
    OP = mybir.AluOpType
    AX = mybir.AxisListType

    x_d = ins["x"]            # [b_core, 120]
    w1_d = ins["w1"]          # [49, NMM]
    w2_d = ins["w2"]          # [94, NMM]
    idn_d = ins["idn"]        # [128, 128]
    hsc_d = ins["hsc"]        # [48]
    epsab_d = ins["epsab"]    # [25]
    out_d = outs["out"]       # [b_core, 600]

    b_core = x_d.shape[0]
    assert b_core % BLOCK == 0
    nblocks = b_core // BLOCK

    def mmcast(ap):
        return ap.bitcast(f32r) if F32R else ap

    from contextlib import ExitStack
    with ExitStack() as ctx:
        consts = ctx.enter_context(tc.tile_pool(name="consts", bufs=1))
        raw_p = ctx.enter_context(tc.tile_pool(name="raw", bufs=3))
        xcv_p = ctx.enter_context(tc.tile_pool(name="xcv", bufs=3))
        s2_p = ctx.enter_context(tc.tile_pool(name="s2", bufs=3))
        scr_p = ctx.enter_context(tc.tile_pool(name="scr", bufs=3))
        it_p = ctx.enter_context(tc.tile_pool(name="it", bufs=3))
        st_p = ctx.enter_context(tc.tile_pool(name="st", bufs=3))
        stg_p = ctx.enter_context(tc.tile_pool(name="stg", bufs=3))
        tps_p = ctx.enter_context(tc.tile_pool(name="tps", bufs=1, space="PSUM"))
        mps_p = ctx.enter_context(tc.tile_pool(name="mps", bufs=3, space="PSUM"))

        # ---- constants into SBUF ----
        mmdt = f32r if F32R else f32
        idn_t = consts.tile([128, 128], f32)
        nc.sync.dma_start(out=idn_t[:], in_=idn_d)
        w1_raw = consts.tile([49, NMM], f32)
        nc.sync.dma_start(out=w1_raw[:], in_=w1_d)
        w2_raw = consts.tile([94, NMM], f32)
        nc.sync.dma_start(out=w2_raw[:], in_=w2_d)
        w1_t = consts.tile([49, NMM], mmdt)
        nc.scalar.copy(out=w1_t[:], in_=w1_raw[:])
        w2_t = consts.tile([94, NMM], mmdt)
        nc.scalar.copy(out=w2_t[:], in_=w2_raw[:])

        def bcast_load(dst, src_ap):
            # DMA-broadcast a [n] dram vector to [128, n] sbuf
            src = bass.AP(tensor=src_ap.tensor, offset=src_ap.offset,
                          ap=[[0, 128]] + [list(p) for p in src_ap.ap])
            nc.sync.dma_start(out=dst, in_=src)

        hsc_t = consts.tile([128, 48], f32)
        bcast_load(hsc_t[:], hsc_d)
        epsab_t = consts.tile([128, 25], f32)
        bcast_load(epsab_t[:], epsab_d)
        epsln_t = consts.tile([128, 1], f32)      # bias tile for Ln(close+EPS)
        nc.vector.memset(epsln_t[:], EPS)

        xr = x_d.rearrange("(blk p g) f -> blk p g f", p=PB, g=G)
        orr = out_d.rearrange("(blk p g) f -> blk p g f", p=PB, g=G)

        def phase1(blk):
            # ---------------- load ----------------
            raw = raw_p.tile([128, G, 120], f32)
            nc.sync.dma_start(out=raw[:], in_=xr[blk])
            rawv = raw.rearrange("p g (s c) -> p g s c", c=CIN)

            # ------------- b-on-P pre-ops (free-axis shifts are free) -----
            # pack cols: close 0:24, vol 24:48, one 48 | LC 49:73,
            #            gain 73:96, loss 96:119, vmult 119:142, one 142
            pk = scr_p.tile([128, G, 143], f32, tag="pk")
            nc.scalar.copy(
                out=pk[:, :, 0:48].rearrange("p g (c s) -> p g c s", c=2),
                in_=rawv[:, :, :, 3:5].rearrange("p g s c -> p g c s"),
            )
            nc.vector.memset(pk[:, :, 48:49], 1.0)
            nc.vector.memset(pk[:, :, 142:143], 1.0)
            nc.scalar.activation(pk[:, :, 49:73], rawv[:, :, :, 3], AF.Ln,
                                 bias=epsln_t[:], scale=1.0)          # LC
            dl = scr_p.tile([128, G, 23], f32, tag="deltas")
            nc.gpsimd.tensor_tensor(dl[:], rawv[:, :, 1:24, 3],
                                    rawv[:, :, 0:23, 3], OP.subtract)
            nc.vector.tensor_scalar_max(pk[:, :, 73:96], dl[:], 0.0)  # gain
            nc.gpsimd.tensor_tensor(pk[:, :, 96:119], pk[:, :, 73:96],
                                    dl[:], OP.subtract)               # loss
            sg = scr_p.tile([128, G, 23], f32, tag="sign")
            nc.scalar.activation(sg[:], dl[:], AF.Sign)
            nc.gpsimd.tensor_tensor(pk[:, :, 119:142], sg[:],
                                    rawv[:, :, 1:24, 4], OP.mult)     # vmult

            # ---------------- in-transposes ----------------
            xcv = xcv_p.tile([49, G * 128], mmdt)  # close s, vol s, ones
            s2 = s2_p.tile([94, G * 128], mmdt)    # LC, gain, loss, vmult, ones
            for w in range(G // 4):
                tp = tps_p.tile([49, 512], f32, tag="tp1")
                tq = tps_p.tile([94, 512], f32, tag="tp2")
                for gg in range(4):
                    g = w * 4 + gg
                    nc.tensor.transpose(
                        tp[:, gg * 128 : (gg + 1) * 128], pk[:, g, 0:49],
                        idn_t[:])
                    nc.tensor.transpose(
                        tq[:, gg * 128 : (gg + 1) * 128], pk[:, g, 49:143],
                        idn_t[:])
                nc.scalar.copy(out=xcv[:, w * 512 : (w + 1) * 512], in_=tp[:])
                nc.scalar.copy(out=s2[:, w * 512 : (w + 1) * 512], in_=tq[:])

            # ---------------- staging + per-block tiles ----------------
            stg = stg_p.tile([128, G, 600], f32)
            stgv = stg.rearrange("p g (s c) -> p g s c", c=COUT)
            it = it_p.tile([128, G, NMM], f32)

            # ---------------- matmuls + psum peel (waves of 2 chunks) ----
            for w in range(G // 2):
                mp = mps_p.tile([128, 2, 512], f32)
                for gg in range(2):
                    j = w * 2 + gg
                    nc.tensor.matmul(mp[:, gg, 0:NMM],
                                     xcv[:, j * 128 : (j + 1) * 128],
                                     w1_t[:], start=True, stop=False)
                    nc.tensor.matmul(mp[:, gg, 0:NMM],
                                     s2[:, j * 128 : (j + 1) * 128],
                                     w2_t[:], start=False, stop=True)
                nc.scalar.copy(out=it[:, 2 * w : 2 * w + 2, :],
                               in_=mp[:, :, 0:NMM])

            if True:
            # ---------------- matmul-final features -> staging (DVE) -----
                nc.vector.tensor_copy(stgv[:, :, :, 5], it[:, :, 240:264])
                nc.vector.tensor_scalar(
                    stgv[:, :, :, 6].bitcast(mybir.dt.int32),
                    it[:, :, 240:264].bitcast(mybir.dt.int32),
                    0x7FFFFFFF, None, OP.bitwise_and)                     # |ret|
                nc.vector.tensor_copy(
                    stgv[:, :, :, 18:20],
                    it[:, :, 264:312].rearrange("p g (s m) -> p g s m", m=2))
                nc.vector.tensor_copy(stgv[:, :, :, 22], it[:, :, 312:336])

                # ---------------- b-on-P nonlinear features ----------------
                c_open = rawv[:, :, :, 0]
                c_high = rawv[:, :, :, 1]
                c_low = rawv[:, :, :, 2]
                c_close = rawv[:, :, :, 3]
                c_vol = rawv[:, :, :, 4]

                # reciprocals via exp(-ln(x)) on ACT (~2 ULP)
                rc = scr_p.tile([128, G, 24], f32, tag="rc")
                nc.scalar.activation(rc[:], pk[:, :, 49:73], AF.Exp, scale=-1.0)
                rsv = scr_p.tile([128, G, 48], f32, tag="rsv")   # rS | rv
                nc.scalar.activation(rsv[:], it[:, :, 192:240], AF.Ln)
                nc.scalar.activation(rsv[:], rsv[:], AF.Exp, scale=-1.0)
                rS = rsv[:, :, 0:24]
                rv_ap = rsv[:, :, 24:48]
                r12 = scr_p.tile([128, G, 24], f32, tag="r12")
                nc.scalar.activation(r12[:], it[:, :, 48:72], AF.Ln)
                nc.scalar.activation(r12[:], r12[:], AF.Exp, scale=-1.0)

                # raw passthrough channels 0..4 + hour consts + zero channels
                nc.gpsimd.tensor_copy(stgv[:, :, :, 0:5], rawv[:, :, :, 0:5])
                hql = hsc_t.rearrange("p (s c) -> p s c", c=2)
                nc.gpsimd.tensor_copy(
                    stgv[:, :, :, 16:18],
                    hql.unsqueeze(1).to_broadcast((128, G, 24, 2)),
                )
                nc.gpsimd.memset(stgv[:, :, :, 23:25], 0.0)

                # ema ratios 9..12 = ema_k_s * rc   (GPSIMD)
                it_ema = it[:, :, 0:96].rearrange("p g (k s) -> p g k s", k=4)
                rc4 = rc.unsqueeze(2).to_broadcast((128, G, 4, 24))
                nc.gpsimd.tensor_tensor(
                    stgv[:, :, :, 9:13].rearrange("p g s c -> p g c s"),
                    it_ema, rc4, OP.mult)
                # vol ratios 13..14
                it_vol = it[:, :, 96:144].rearrange("p g (k s) -> p g k s", k=2)
                rv2 = rv_ap.unsqueeze(2).to_broadcast((128, G, 2, 24))
                nc.gpsimd.tensor_tensor(
                    stgv[:, :, :, 13:15].rearrange("p g s c -> p g c s"),
                    it_vol, rv2, OP.mult)
                # ch20 = close * r12  (the -1 shift is normalization-invariant)
                nc.vector.tensor_tensor(stgv[:, :, :, 20], c_close, r12[:], OP.mult)
                # ch21 = u = (avgL+eps)*rS   (rsi = 100-100u; sign via inv)
                nc.vector.tensor_tensor(stgv[:, :, :, 21], it[:, :, 168:192],
                                        rS, OP.mult)
                # ch7 = (high-low)*rc
                hl = scr_p.tile([128, G, 24], f32, tag="hl")
                nc.gpsimd.tensor_tensor(hl[:], c_high, c_low, OP.subtract)
                nc.gpsimd.tensor_tensor(stgv[:, :, :, 7], hl[:], rc[:], OP.mult)
                # ch8 = |open-close|*rc
                oc = scr_p.tile([128, G, 24], f32, tag="oc")
                i32 = mybir.dt.int32
                nc.vector.tensor_tensor(oc[:], c_open, c_close, OP.subtract)
                nc.vector.tensor_scalar(oc[:].bitcast(i32), oc[:].bitcast(i32),
                                        0x7FFFFFFF, None, OP.bitwise_and)
                nc.gpsimd.tensor_tensor(stgv[:, :, :, 8], oc[:], rc[:], OP.mult)
                # ch15 = log1p(vol)
                nc.scalar.activation(stgv[:, :, :, 15], c_vol, AF.Ln, bias=1.0)


            return dict(stg=stg, stgv=stgv, it=it, raw=raw, rawv=rawv)

        def phase2(blk, t):
            stg, stgv, it, raw, rawv = t['stg'], t['stgv'], t['it'], t['raw'], t['rawv']
            if True:
            # ---------------- stats: sums + sum of squares ----------------
                sums = st_p.tile([128, G, COUT], f32, tag="sums")
                nc.vector.reduce_sum(sums[:], stgv.rearrange("p g s c -> p g c s"),
                                     axis=AX.X)
                sumsq = st_p.tile([128, G, COUT], f32, tag="sumsq")
                QG = G // 2
                for q in range(G // QG):
                    fsq = st_p.tile([128, QG, 600], f32, tag="fsq")
                    sl = stg[:, q * QG : (q + 1) * QG, :]
                    nc.scalar.activation(fsq[:], sl, AF.Square)
                    nc.vector.reduce_sum(
                        sumsq[:, q * QG : (q + 1) * QG, :],
                        fsq.rearrange("p q (s c) -> p q c s", c=COUT),
                        axis=AX.X)

                # negm = -sums/24; var*24 + 24*eps' = sumsq + negm*sums + 24*eps'
                negm = st_p.tile([128, G, COUT], f32, tag="negm")
                nc.gpsimd.tensor_scalar_mul(negm[:], sums[:], -1.0 / 24.0)
                m2t = st_p.tile([128, G, COUT], f32, tag="m2t")
                nc.gpsimd.tensor_tensor(m2t[:], negm[:], sums[:], OP.mult)
                nc.gpsimd.tensor_tensor(m2t[:], sumsq[:], m2t[:], OP.add)
                nc.gpsimd.tensor_tensor(
                    m2t[:], m2t[:],
                    epsab_t[:].unsqueeze(1).to_broadcast((128, G, COUT)), OP.add)
                # inv = rsqrt(var + eps') = exp(-0.5*ln(m2t/24))
                inv = st_p.tile([128, G, COUT], f32, tag="inv")
                nc.scalar.activation(inv[:], m2t[:], AF.Ln, scale=1.0 / 24.0)
                nc.scalar.activation(inv[:], inv[:], AF.Exp, scale=-0.5)
                # rsi channel: out = -norm(u)
                nc.vector.tensor_scalar_mul(inv[:, :, 21:22], inv[:, :, 21:22], -1.0)

                # ---------------- affine normalize (in place on staging) -----
                def affine(eng, c0, c1):
                    tgt = stgv[:, :, :, c0:c1]
                    nch = c1 - c0
                    msb = negm[:, :, c0:c1].unsqueeze(2).to_broadcast(
                        (128, G, 24, nch))
                    invb = inv[:, :, c0:c1].unsqueeze(2).to_broadcast(
                        (128, G, 24, nch))
                    eng.tensor_tensor(tgt, tgt, msb, OP.add)
                    eng.tensor_tensor(tgt, tgt, invb, OP.mult)

                affine(nc.vector, 0, 11)
                affine(nc.gpsimd, 11, 16)
                affine(nc.gpsimd, 18, 23)

            # ---------------- store ----------------
            nc.sync.dma_start(out=orr[blk], in_=stg[:])



        seq = [b for _ in range(repeat) for b in range(nblocks)]
        pend = []
        for blk in seq:
            t = phase1(blk)
            pend.append((blk, t))
            if len(pend) > 1:
                phase2(*pend.pop(0))
        for p in pend:
            phase2(*p)

# --------------------------------------------------------------------------
# host wrapper
# --------------------------------------------------------------------------

_CACHE = {}


def _compiled(b_core, repeat=1):
    if (b_core, repeat) in _CACHE:
        return _CACHE[(b_core, repeat)]
    import concourse.bacc as bacc
    import concourse.tile as tile
    from concourse import mybir, hw_specs

    f32 = mybir.dt.float32
    nc = bacc.Bacc("TRN2", target_bir_lowering=False, debug=False)
    ins = {
        "x": nc.dram_tensor("x", [b_core, 120], f32, kind="ExternalInput").ap(),
        "w1": nc.dram_tensor("w1", [49, NMM], f32, kind="ExternalInput").ap(),
        "w2": nc.dram_tensor("w2", [94, NMM], f32, kind="ExternalInput").ap(),
        "idn": nc.dram_tensor("idn", [128, 128], f32, kind="ExternalInput").ap(),
        "hsc": nc.dram_tensor("hsc", [48], f32, kind="ExternalInput").ap(),
        "epsab": nc.dram_tensor("epsab", [25], f32, kind="ExternalInput").ap(),
    }
    outs = {
        "out": nc.dram_tensor("out", [b_core, 600], f32,
                              kind="ExternalOutput").ap(),
    }
    with tile.TileContext(nc) as tc:
        kernel_body(tc, outs, ins, repeat=repeat)

    # Pin every activation to one table set so the compiler emits a single
    # ACT_TABLE_LOAD instead of thrashing between per-function sets.
    tables = hw_specs.get_activation_tables(nc.m.arch)
    saved = {k: set(v) for k, v in tables.items()}
    try:
        for k in tables:
            if k != ACT_SET:
                tables[k] = set()
        nc.compile()
    finally:
        for k, v in saved.items():
            tables[k] = v
    _CACHE[(b_core, repeat)] = nc
    return nc


def kernel(x):
    from concourse import bass_utils

    x = np.ascontiguousarray(np.asarray(x, dtype=np.float32))
    B = x.shape[0]
    assert B % NCORES == 0
    b_core = B // NCORES
    consts = _consts()
    nc = _compiled(b_core)
    xf = x.reshape(B, S * CIN)
    in_maps = [
        {"x": np.ascontiguousarray(xf[i * b_core : (i + 1) * b_core]), **consts}
        for i in range(NCORES)
    ]
    res = bass_utils.run_bass_kernel_spmd(nc, in_maps, core_ids=list(range(NCORES)))
    out = np.concatenate([r["out"] for r in res.results], axis=0)
    return out.reshape(B, S, COUT)

